# revision 11
# baseline (speedup 1.0000x reference)
"""Trainium2 Bass kernel for nn_AssignAttention (hard-assignment MoE-routing attention).

Math (forward): for each (b, h, key-token s), the key token is hard-assigned to
group n* = argmax_n (q_bhn . k_bhs); output per group = sum of assigned v vectors
scaled by 1/(count+1), then projected.  The straight-through softmax terms cancel
in forward up to ~1e-7, so only the argmax routing matters.

Strategy (v3):
 - Pure data-parallel over batch B=16 across 8 cores (2 batches/core), no collectives.
 - Host precomputes t[b,h,n,:] = Wk_h^T Wq_h query[b,n] so attention logits are
   attn[s, (h,n)] = key[b,s,:] . t[b,h,n,:]  -- one C-contraction against raw key
   in float32r (argmax routing needs ~13-bit logit precision; bf16 flips too many
   near-ties).
 - KEY INSIGHT vs v2: the group-sum is linear, so sum_{s in G} v[s] =
   (sum_{s in G} key[s]) @ Wv^T.  The device scatters RAW key vectors (shipped a
   second time as bf16 in natural [S, C] layout, which is exactly the rhs layout
   the scatter matmul needs) and the HOST applies Wv/Wp afterwards.  This deletes
   the entire v-projection matmul pipeline (1/3 of PE work) and the per-subtile
   PSUM->SBUF v copy; the scatter rhs is wider (385 vs 130) but that costs less
   than the v matmuls did.  bf16-rounding raw key costs the same error as the old
   bf16 v65 rounding did.
 - Per 256-row s-pair (2 subtiles): 6 attn matmuls -> one paired argmax
   (reduce_max) + one paired is_equal -> bf16 one-hot for both subtiles (pairing
   halves DVE fixed overhead; DVE is the co-critical engine).  Scatter:
   o[:, p, :] += aT_pair^T @ [key_bf16 | 1] per subtile and head-pair p; the ones
   column yields per-group counts.  o-matmuls of pair i issue after pair i+1's
   attn matmuls so the PE never waits on DVE.
 - Epilogue (1/(cnt+1) scaling, head unpack, Wv+Wp projection, bias) runs on
   HOST from the raw DMA'd accumulator.
 - Startup: DMA issue on the Sync engine costs ~650ns/instruction, so the first
   dependencies (tc ct0, first 128 key columns) are issued first and the rest
   follow; PE warmup matmuls open the HAM clock gate during the wait.
"""
import sys

sys.path.insert(0, "/opt/trn_rl_repo")

import numpy as np
import ml_dtypes

import concourse.bass as bass
import concourse.mybir as mybir
import concourse.tile as tile
from concourse.bass_utils import run_bass_kernel_spmd

B, N, S, C, H = 16, 64, 4096, 384, 6
DH = C // H  # 64
NCORES = 8
BPC = B // NCORES  # batches per core = 2
CT = C // 128  # c-tiles = 3
S_CHUNK = 512
N_CHUNKS = S // S_CHUNK  # 8
PAIRS = S_CHUNK // 256  # 2 s-pairs per chunk
KW = C + 1  # scatter rhs width (key + ones col) = 385
KWP = 390  # padded bf16 key tile width
OW = 512  # padded o accumulator width per head-pair (one PSUM bank)

F32 = mybir.dt.float32
F32R = mybir.dt.float32r
BF16 = mybir.dt.bfloat16

LAST_RESULT = None  # stash of BassKernelResults for profiling in test.py


def _split_multiwaits(nc):
    """walrus codegen in this toolchain accepts at most one sync-wait per
    instruction; hoist extras onto standalone wait-only EventSemaphore
    instructions placed immediately before (same engine, so ordering holds)."""
    for fn in nc.m.functions:
        for blk in fn.blocks:
            new = []
            for inst in blk.instructions:
                si = inst.sync_info
                if si is not None and si.on_wait and len(si.on_wait) > 1:
                    for w in si.on_wait[:-1]:
                        ev = mybir.InstEventSemaphore(
                            name=nc.get_next_instruction_name(), ins=[], outs=[]
                        )
                        ev.engine = inst.engine
                        ev.sync_info = mybir.SyncInfo(on_wait=[w], on_update=[])
                        new.append(ev)
                    inst.sync_info = mybir.SyncInfo(
                        on_wait=[si.on_wait[-1]], on_update=si.on_update
                    )
                new.append(inst)
            blk.instructions = new


def _build_kernel():
    nc = bass.Bass()
    keyT_d = nc.declare_dram_parameter("keyT", [BPC, C, S], F32R, isOutput=False)
    keyn_d = nc.declare_dram_parameter("keyn", [BPC, S, C], BF16, isOutput=False)
    tc_d = nc.declare_dram_parameter("tc", [BPC, C, C], F32R, isOutput=False)
    out_d = nc.declare_dram_parameter("out", [BPC, 128, CT, KW], F32, isOutput=True)

    with tile.TileContext(nc) as tc:
        with (
            tc.tile_pool(name="consts", bufs=1) as consts,
            tc.tile_pool(name="perb", bufs=2) as perb,
            tc.tile_pool(name="keyp", bufs=4) as keyp,
            tc.tile_pool(name="work", bufs=4) as work,
            tc.tile_pool(name="epi", bufs=2) as epi,
            tc.tile_pool(name="ps_attn", bufs=2, space="PSUM") as ps_attn,
            tc.tile_pool(name="ps_o", bufs=1, space="PSUM") as ps_o,
        ):
            # startup DMAs, ordered so the first attn matmul's deps land first:
            # tc ct0 -> kt cols 0:128 -> tc ct1/ct2 -> kt cols 128:512 -> key_nat
            tc_first = perb.tile([128, CT, C], F32R, tag="tc_sb")
            tc_r0 = tc_d[0].rearrange("(ct p) hn -> p ct hn", p=128)
            nc.sync.dma_start(out=tc_first[:, 0, :], in_=tc_r0[:, 0, :])
            kt_first = keyp.tile([128, CT, S_CHUNK], F32R, tag="kt")
            keyT_r0 = keyT_d[0].rearrange("(ct p) s -> p ct s", p=128)
            nc.sync.dma_start(
                out=kt_first[:, :, 0:256], in_=keyT_r0[:, :, 0:256]
            )
            nc.sync.dma_start(out=tc_first[:, 1, :], in_=tc_r0[:, 1, :])
            nc.sync.dma_start(out=tc_first[:, 2, :], in_=tc_r0[:, 2, :])
            nc.sync.dma_start(
                out=kt_first[:, :, 256:S_CHUNK], in_=keyT_r0[:, :, 256:S_CHUNK]
            )

            # persistent bf16 natural-layout key ring ([s_part, 4 subtiles, C+1];
            # the ones column at col C is preset once and never re-written: the
            # chunk DMAs only fill cols 0:C)
            NKR = 3
            kn_ring = [
                consts.tile([128, 4, KWP], BF16, name=f"kn_{i}") for i in range(NKR)
            ]
            for t in kn_ring:
                nc.gpsimd.memset(t[:, :, C : C + 1], 1.0)

            # PE warmup: back-to-back matmuls on scratch during the initial DMA
            # wait, so the HAM clock-gate reaches 8/8 before real work arrives.
            # The scratch PSUM comes from the ps_o pool (it rotates into the
            # real o accumulator, whose memzero follows the warmup anyway).
            warm_sb = consts.tile([128, 640], BF16)
            nc.gpsimd.memset(warm_sb[:], 0.0)
            warm_ps = ps_o.tile([128, CT, OW], F32, tag="o_ps")
            for _ in range(8):
                nc.tensor.matmul(
                    warm_ps[:, 0, :], warm_sb[:, 0:128], warm_sb[:, 128:640],
                    start=True, stop=True,
                )

            for b in range(BPC):
                if b == 0:
                    tc_sb = tc_first
                else:
                    tc_sb = perb.tile([128, CT, C], F32R, tag="tc_sb")
                    nc.sync.dma_start(
                        out=tc_sb[:],
                        in_=tc_d[b].rearrange("(ct p) hn -> p ct hn", p=128),
                    )
                # per-group accumulator, head-PAIR packed: for pair p, partition
                # rows 0..63 = head 2p groups, rows 64..127 = head 2p+1 groups;
                # cols 0..C-1 = raw-key sums, col C = counts (both heads).
                # Zeroed on ACT (keeps DVE free); the accumulating matmuls use
                # start=False so their scheduling order doesn't matter.
                o_ps = ps_o.tile([128, CT, OW], F32)
                nc.scalar.memzero(o_ps[:, :, 0:KW])

                keyT_b = keyT_d[b].rearrange("(ct p) s -> p ct s", p=128)
                # software pipeline: issue s-pair i's o-matmuls AFTER pair
                # i+1's attn matmuls, so the PE never waits on DVE's one-hot.
                pending = None  # (aT2, kn_sb, pair_in_chunk) of previous s-pair

                def flush_o(stop):
                    aT2_p, kn_p, pr = pending
                    for k in range(2):
                        for p in range(CT):
                            nc.tensor.matmul(
                                o_ps[:, p, 0:KW],
                                aT2_p[:, k]
                                .rearrange("q h n -> q (h n)")[
                                    :, 2 * p * N : (2 * p + 2) * N
                                ],
                                kn_p[:, 2 * pr + k, 0:KW],
                                start=False,
                                stop=stop and k == 1,
                                skip_group_check=True,
                            )

                for ch in range(N_CHUNKS):
                    if b == 0 and ch == 0:
                        kt_sb = kt_first
                    else:
                        kt_sb = keyp.tile([128, CT, S_CHUNK], F32R, tag="kt")
                        nc.sync.dma_start(
                            out=kt_sb[:],
                            in_=keyT_b[:, :, ch * S_CHUNK : (ch + 1) * S_CHUNK],
                        )
                    # kn DMAs issue from the (otherwise idle) Scalar queue so
                    # they don't serialize behind kt issues on Sync
                    kn_sb = kn_ring[(b * N_CHUNKS + ch) % NKR]
                    nc.scalar.dma_start(
                        out=kn_sb[:, :, 0:C],
                        in_=keyn_d[b, ch * S_CHUNK : (ch + 1) * S_CHUNK, :].rearrange(
                            "(c4 p) c -> p c4 c", p=128
                        ),
                    )
                    for pr in range(PAIRS):
                        attn2 = ps_attn.tile([128, 2, OW], F32)
                        # per-subtile reduce_max (issued right after each
                        # subtile's logits close) + ONE paired is_equal: the
                        # split reduce lets DVE start ~0.5us earlier, so the
                        # PSUM tile frees before the next-next pair needs it.
                        gmax2 = work.tile([128, 2, H], F32)
                        for k in range(2):
                            sl = slice(
                                (2 * pr + k) * 128, (2 * pr + k) * 128 + 128
                            )
                            for ct in range(CT):
                                nc.tensor.matmul(
                                    attn2[:, k, 0:C],
                                    kt_sb[:, ct, sl],
                                    tc_sb[:, ct, :],
                                    start=(ct == 0),
                                    stop=(ct == CT - 1),
                                )
                            nc.vector.reduce_max(
                                out=gmax2[:, k : k + 1, :],
                                in_=attn2[:, k : k + 1, 0:C].rearrange(
                                    "p k (h n) -> p k h n", h=H
                                ),
                                axis=mybir.AxisListType.X,
                            )
                        if pending is not None:
                            flush_o(stop=False)
                        aT2 = work.tile([128, 2, H, N], BF16)
                        g = gmax2[:]
                        g_bcast = bass.AP(
                            tensor=g.tensor, offset=g.offset,
                            ap=[g.ap[0], g.ap[1], g.ap[2], [0, N]],
                        )
                        nc.vector.tensor_tensor(
                            out=aT2[:],
                            in0=attn2[:, :, 0:C].rearrange(
                                "p k (h n) -> p k h n", h=H
                            ),
                            in1=g_bcast,
                            op=mybir.AluOpType.is_equal,
                        )
                        pending = (aT2, kn_sb, pr)
                flush_o(stop=True)
                pending = None
                # per-batch tail: raw accumulator -> SBUF -> DRAM; the scaling,
                # head unpack, and Wv/Wp projections happen on host.  Copies go
                # per head-pair so they pipeline under the final o-matmuls.
                o_sb = epi.tile([128, CT, KW], F32)
                for p in range(CT):
                    nc.scalar.copy(out=o_sb[:, p, :], in_=o_ps[:, p, 0:KW])
                nc.sync.dma_start(out=out_d[b], in_=o_sb[:])

    _split_multiwaits(nc)
    return nc


_NC_CACHE = None


def _get_nc():
    global _NC_CACHE
    if _NC_CACHE is None:
        _NC_CACHE = _build_kernel()
    return _NC_CACHE


def kernel(query, key, Wq, Wk, Wv, Wp, bp):
    global LAST_RESULT
    query = np.ascontiguousarray(query, dtype=np.float32)
    key = np.ascontiguousarray(key, dtype=np.float32)
    Wq = np.asarray(Wq, dtype=np.float32)
    Wk = np.asarray(Wk, dtype=np.float32)
    Wv = np.asarray(Wv, dtype=np.float32)
    Wp = np.asarray(Wp, dtype=np.float32)
    bp = np.asarray(bp, dtype=np.float32)

    # host prep: t[b,h,n,:] = Wk_h^T Wq_h query[b,n]  (tiny; never touches `key`)
    q = query @ Wq.T  # [B, N, C]
    qh = q.reshape(B, N, H, DH).transpose(0, 2, 1, 3)  # [B,H,N,DH]
    Wk_h = Wk.reshape(H, DH, C)
    t = np.einsum("bhnd,hdc->bhnc", qh, Wk_h)  # [B,H,N,C]
    # Tc[b] layout: [C, (h n)] with column h*N+n = t[b,h,n,:]
    Tc = np.ascontiguousarray(
        t.transpose(0, 3, 1, 2).reshape(B, C, H * N), dtype=np.float32
    )
    keyT = np.ascontiguousarray(key.transpose(0, 2, 1), dtype=np.float32)  # [B,C,S]
    keyn = np.ascontiguousarray(key).astype(ml_dtypes.bfloat16)  # [B,S,C]

    nc = _get_nc()
    in_maps = [
        {
            "keyT": keyT[i * BPC : (i + 1) * BPC],
            "keyn": keyn[i * BPC : (i + 1) * BPC],
            "tc": Tc[i * BPC : (i + 1) * BPC],
        }
        for i in range(NCORES)
    ]
    try:
        res = run_bass_kernel_spmd(nc, in_maps, core_ids=list(range(NCORES)))
    except Exception:
        # transient NRT device errors have been observed; retry once
        res = run_bass_kernel_spmd(nc, in_maps, core_ids=list(range(NCORES)))
    LAST_RESULT = res
    o = np.concatenate([res.results[i]["out"] for i in range(NCORES)], axis=0)
    # o: [B, 128, CT, C+1] head-pair-packed raw-key group sums + counts.
    # Host epilogue: unpack heads, scale by 1/(cnt+1), apply Wv then Wp.
    cnt = o[:, :, :, C]  # [B, 128, CT]
    scale = 1.0 / (cnt + 1.0)
    r0 = o[:, 0:N, :, 0:C] * scale[:, 0:N, :, None]  # [B, n, p, c] heads 2p
    r1 = o[:, N:128, :, 0:C] * scale[:, N:128, :, None]  # heads 2p+1
    r = np.empty((B, N, H, C), np.float32)
    r[:, :, 0::2, :] = r0
    r[:, :, 1::2, :] = r1
    Wv_h = Wv.reshape(H, DH, C)
    vsum = np.einsum("bnhc,hdc->bnhd", r, Wv_h)  # [B, N, H, DH]
    out = vsum.reshape(B, N, C) @ Wp.T + bp
    return out.astype(np.float32)


# revision 12
# speedup vs baseline: 1.3401x; 1.3401x over previous
"""Trainium2 Bass kernel for nn_AssignAttention (hard-assignment MoE-routing attention).

Math (forward): for each (b, h, key-token s), the key token is hard-assigned to
group n* = argmax_n (q_bhn . k_bhs); output per group = sum of assigned v vectors
scaled by 1/(count+1), then projected.  The straight-through softmax terms cancel
in forward up to ~1e-7, so only the argmax routing matters.

Strategy (v3):
 - Pure data-parallel over batch B=16 across 8 cores (2 batches/core), no collectives.
 - Host precomputes t[b,h,n,:] = Wk_h^T Wq_h query[b,n] so attention logits are
   attn[s, (h,n)] = key[b,s,:] . t[b,h,n,:]  -- one C-contraction against raw key
   in float32r (argmax routing needs ~13-bit logit precision; bf16 flips too many
   near-ties).
 - KEY INSIGHT vs v2: the group-sum is linear, so sum_{s in G} v[s] =
   (sum_{s in G} key[s]) @ Wv^T.  The device scatters RAW key vectors (shipped a
   second time as bf16 in natural [S, C] layout, which is exactly the rhs layout
   the scatter matmul needs) and the HOST applies Wv/Wp afterwards.  This deletes
   the entire v-projection matmul pipeline (1/3 of PE work) and the per-subtile
   PSUM->SBUF v copy; the scatter rhs is wider (385 vs 130) but that costs less
   than the v matmuls did.  bf16-rounding raw key costs the same error as the old
   bf16 v65 rounding did.
 - Per 256-row s-pair (2 subtiles): 6 attn matmuls -> one paired argmax
   (reduce_max) + one paired is_equal -> bf16 one-hot for both subtiles (pairing
   halves DVE fixed overhead; DVE is the co-critical engine).  Scatter:
   o[:, p, :] += aT_pair^T @ [key_bf16 | 1] per subtile and head-pair p; the ones
   column yields per-group counts.  o-matmuls of pair i issue after pair i+1's
   attn matmuls so the PE never waits on DVE.
 - Epilogue (1/(cnt+1) scaling, head unpack, Wv+Wp projection, bias) runs on
   HOST from the raw DMA'd accumulator.
 - Startup: DMA issue on the Sync engine costs ~650ns/instruction, so the first
   dependencies (tc ct0, first 128 key columns) are issued first and the rest
   follow; PE warmup matmuls open the HAM clock gate during the wait.
"""
import sys

sys.path.insert(0, "/opt/trn_rl_repo")

import numpy as np
import ml_dtypes

import concourse.bass as bass
import concourse.mybir as mybir
import concourse.tile as tile
from concourse.bass_utils import run_bass_kernel_spmd

B, N, S, C, H = 16, 64, 4096, 384, 6
DH = C // H  # 64
NCORES = 8
BPC = B // NCORES  # batches per core = 2
CT = C // 128  # c-tiles = 3
S_CHUNK = 512
N_CHUNKS = S // S_CHUNK  # 8
PAIRS = S_CHUNK // 256  # 2 s-pairs per chunk
KW = C + 1  # scatter rhs width (key + ones col) = 385
KWP = 390  # padded bf16 key tile width
OW = 512  # padded o accumulator width per head-pair (one PSUM bank)

F32 = mybir.dt.float32
F32R = mybir.dt.float32r
BF16 = mybir.dt.bfloat16

LAST_RESULT = None  # stash of BassKernelResults for profiling in test.py


def _split_multiwaits(nc):
    """walrus codegen in this toolchain accepts at most one sync-wait per
    instruction; hoist extras onto standalone wait-only EventSemaphore
    instructions placed immediately before (same engine, so ordering holds)."""
    for fn in nc.m.functions:
        for blk in fn.blocks:
            new = []
            for inst in blk.instructions:
                si = inst.sync_info
                if si is not None and si.on_wait and len(si.on_wait) > 1:
                    for w in si.on_wait[:-1]:
                        ev = mybir.InstEventSemaphore(
                            name=nc.get_next_instruction_name(), ins=[], outs=[]
                        )
                        ev.engine = inst.engine
                        ev.sync_info = mybir.SyncInfo(on_wait=[w], on_update=[])
                        new.append(ev)
                    inst.sync_info = mybir.SyncInfo(
                        on_wait=[si.on_wait[-1]], on_update=si.on_update
                    )
                new.append(inst)
            blk.instructions = new


def _build_kernel():
    nc = bass.Bass()
    keyT_d = nc.declare_dram_parameter("keyT", [BPC, C, S], F32R, isOutput=False)
    keyn_d = nc.declare_dram_parameter("keyn", [BPC, S, C], BF16, isOutput=False)
    tc_d = nc.declare_dram_parameter("tc", [BPC, C, C], F32R, isOutput=False)
    out_d = nc.declare_dram_parameter("out", [BPC, 128, CT, KW], F32, isOutput=True)

    with tile.TileContext(nc) as tc:
        with (
            tc.tile_pool(name="consts", bufs=1) as consts,
            tc.tile_pool(name="perb", bufs=2) as perb,
            tc.tile_pool(name="keyp", bufs=4) as keyp,
            tc.tile_pool(name="work", bufs=4) as work,
            tc.tile_pool(name="epi", bufs=2) as epi,
            tc.tile_pool(name="ps_attn", bufs=2, space="PSUM") as ps_attn,
            tc.tile_pool(name="ps_o", bufs=1, space="PSUM") as ps_o,
        ):
            # startup DMAs, ordered so the first attn matmul's deps land first:
            # tc ct0 -> kt cols 0:128 -> tc ct1/ct2 -> kt cols 128:512 -> key_nat
            tc_first = perb.tile([128, CT, C], F32R, tag="tc_sb")
            tc_r0 = tc_d[0].rearrange("(ct p) hn -> p ct hn", p=128)
            nc.sync.dma_start(out=tc_first[:, 0, :], in_=tc_r0[:, 0, :])
            kt_first = keyp.tile([128, CT, S_CHUNK], F32R, tag="kt")
            keyT_r0 = keyT_d[0].rearrange("(ct p) s -> p ct s", p=128)
            nc.sync.dma_start(
                out=kt_first[:, :, 0:256], in_=keyT_r0[:, :, 0:256]
            )
            nc.sync.dma_start(out=tc_first[:, 1, :], in_=tc_r0[:, 1, :])
            nc.sync.dma_start(out=tc_first[:, 2, :], in_=tc_r0[:, 2, :])
            nc.sync.dma_start(
                out=kt_first[:, :, 256:S_CHUNK], in_=keyT_r0[:, :, 256:S_CHUNK]
            )

            # persistent bf16 natural-layout key ring ([s_part, 4 subtiles, C+1];
            # the ones column at col C is preset once and never re-written: the
            # chunk DMAs only fill cols 0:C)
            NKR = 3
            kn_ring = [
                consts.tile([128, 4, KWP], BF16, name=f"kn_{i}") for i in range(NKR)
            ]
            for t in kn_ring:
                nc.gpsimd.memset(t[:, :, C : C + 1], 1.0)

            # PE warmup: back-to-back matmuls on scratch during the initial DMA
            # wait, so the HAM clock-gate reaches 8/8 before real work arrives.
            # The scratch PSUM comes from the ps_o pool (it rotates into the
            # real o accumulator, whose memzero follows the warmup anyway).
            warm_sb = consts.tile([128, 640], BF16)
            nc.gpsimd.memset(warm_sb[:], 0.0)
            warm_ps = ps_o.tile([128, CT, OW], F32, tag="o_ps")
            for _ in range(8):
                nc.tensor.matmul(
                    warm_ps[:, 0, :], warm_sb[:, 0:128], warm_sb[:, 128:640],
                    start=True, stop=True,
                )

            for b in range(BPC):
                if b == 0:
                    tc_sb = tc_first
                else:
                    tc_sb = perb.tile([128, CT, C], F32R, tag="tc_sb")
                    nc.sync.dma_start(
                        out=tc_sb[:],
                        in_=tc_d[b].rearrange("(ct p) hn -> p ct hn", p=128),
                    )
                # per-group accumulator, head-PAIR packed: for pair p, partition
                # rows 0..63 = head 2p groups, rows 64..127 = head 2p+1 groups;
                # cols 0..C-1 = raw-key sums, col C = counts (both heads).
                # Zeroed on ACT (keeps DVE free); the accumulating matmuls use
                # start=False so their scheduling order doesn't matter.
                o_ps = ps_o.tile([128, CT, OW], F32)
                nc.scalar.memzero(o_ps[:, :, 0:KW])

                keyT_b = keyT_d[b].rearrange("(ct p) s -> p ct s", p=128)
                # software pipeline: issue s-pair i's o-matmuls AFTER pair
                # i+1's attn matmuls, so the PE never waits on DVE's one-hot.
                pending = None  # (aT2, kn_sb, pair_in_chunk) of previous s-pair

                def flush_o(stop):
                    aT2_p, kn_p, pr = pending
                    for k in range(2):
                        for p in range(CT):
                            nc.tensor.matmul(
                                o_ps[:, p, 0:KW],
                                aT2_p[:, k]
                                .rearrange("q h n -> q (h n)")[
                                    :, 2 * p * N : (2 * p + 2) * N
                                ],
                                kn_p[:, 2 * pr + k, 0:KW],
                                start=False,
                                stop=stop and k == 1,
                                skip_group_check=True,
                            )

                for ch in range(N_CHUNKS):
                    if b == 0 and ch == 0:
                        kt_sb = kt_first
                    else:
                        kt_sb = keyp.tile([128, CT, S_CHUNK], F32R, tag="kt")
                        nc.sync.dma_start(
                            out=kt_sb[:],
                            in_=keyT_b[:, :, ch * S_CHUNK : (ch + 1) * S_CHUNK],
                        )
                    # kn DMAs issue from the (otherwise idle) Scalar queue so
                    # they don't serialize behind kt issues on Sync
                    kn_sb = kn_ring[(b * N_CHUNKS + ch) % NKR]
                    nc.scalar.dma_start(
                        out=kn_sb[:, :, 0:C],
                        in_=keyn_d[b, ch * S_CHUNK : (ch + 1) * S_CHUNK, :].rearrange(
                            "(c4 p) c -> p c4 c", p=128
                        ),
                    )
                    for pr in range(PAIRS):
                        attn2 = ps_attn.tile([128, 2, OW], F32)
                        for k in range(2):
                            sl = slice(
                                (2 * pr + k) * 128, (2 * pr + k) * 128 + 128
                            )
                            for ct in range(CT):
                                nc.tensor.matmul(
                                    attn2[:, k, 0:C],
                                    kt_sb[:, ct, sl],
                                    tc_sb[:, ct, :],
                                    start=(ct == 0),
                                    stop=(ct == CT - 1),
                                )
                        if pending is not None:
                            flush_o(stop=False)
                        # paired per-head argmax -> one-hot (bf16)
                        gmax2 = work.tile([128, 2, H], F32)
                        nc.vector.reduce_max(
                            out=gmax2[:],
                            in_=attn2[:, :, 0:C].rearrange(
                                "p k (h n) -> p k h n", h=H
                            ),
                            axis=mybir.AxisListType.X,
                        )
                        aT2 = work.tile([128, 2, H, N], BF16)
                        g = gmax2[:]
                        g_bcast = bass.AP(
                            tensor=g.tensor, offset=g.offset,
                            ap=[g.ap[0], g.ap[1], g.ap[2], [0, N]],
                        )
                        nc.vector.tensor_tensor(
                            out=aT2[:],
                            in0=attn2[:, :, 0:C].rearrange(
                                "p k (h n) -> p k h n", h=H
                            ),
                            in1=g_bcast,
                            op=mybir.AluOpType.is_equal,
                        )
                        pending = (aT2, kn_sb, pr)
                flush_o(stop=True)
                pending = None
                # per-batch tail: raw accumulator -> SBUF -> DRAM; the scaling,
                # head unpack, and Wv/Wp projections happen on host.  Copies go
                # per head-pair so they pipeline under the final o-matmuls.
                o_sb = epi.tile([128, CT, KW], F32)
                for p in range(CT):
                    nc.scalar.copy(out=o_sb[:, p, :], in_=o_ps[:, p, 0:KW])
                nc.sync.dma_start(out=out_d[b], in_=o_sb[:])

    _split_multiwaits(nc)
    return nc


_NC_CACHE = None


def _get_nc():
    global _NC_CACHE
    if _NC_CACHE is None:
        _NC_CACHE = _build_kernel()
    return _NC_CACHE


def kernel(query, key, Wq, Wk, Wv, Wp, bp):
    global LAST_RESULT
    query = np.ascontiguousarray(query, dtype=np.float32)
    key = np.ascontiguousarray(key, dtype=np.float32)
    Wq = np.asarray(Wq, dtype=np.float32)
    Wk = np.asarray(Wk, dtype=np.float32)
    Wv = np.asarray(Wv, dtype=np.float32)
    Wp = np.asarray(Wp, dtype=np.float32)
    bp = np.asarray(bp, dtype=np.float32)

    # host prep: t[b,h,n,:] = Wk_h^T Wq_h query[b,n]  (tiny; never touches `key`)
    q = query @ Wq.T  # [B, N, C]
    qh = q.reshape(B, N, H, DH).transpose(0, 2, 1, 3)  # [B,H,N,DH]
    Wk_h = Wk.reshape(H, DH, C)
    t = np.einsum("bhnd,hdc->bhnc", qh, Wk_h)  # [B,H,N,C]
    # Tc[b] layout: [C, (h n)] with column h*N+n = t[b,h,n,:]
    Tc = np.ascontiguousarray(
        t.transpose(0, 3, 1, 2).reshape(B, C, H * N), dtype=np.float32
    )
    keyT = np.ascontiguousarray(key.transpose(0, 2, 1), dtype=np.float32)  # [B,C,S]
    keyn = np.ascontiguousarray(key).astype(ml_dtypes.bfloat16)  # [B,S,C]

    nc = _get_nc()
    in_maps = [
        {
            "keyT": keyT[i * BPC : (i + 1) * BPC],
            "keyn": keyn[i * BPC : (i + 1) * BPC],
            "tc": Tc[i * BPC : (i + 1) * BPC],
        }
        for i in range(NCORES)
    ]
    try:
        res = run_bass_kernel_spmd(nc, in_maps, core_ids=list(range(NCORES)))
    except Exception:
        # transient NRT device errors have been observed; retry once
        res = run_bass_kernel_spmd(nc, in_maps, core_ids=list(range(NCORES)))
    LAST_RESULT = res
    o = np.concatenate([res.results[i]["out"] for i in range(NCORES)], axis=0)
    # o: [B, 128, CT, C+1] head-pair-packed raw-key group sums + counts.
    # Host epilogue: unpack heads, scale by 1/(cnt+1), apply Wv then Wp.
    cnt = o[:, :, :, C]  # [B, 128, CT]
    scale = 1.0 / (cnt + 1.0)
    r0 = o[:, 0:N, :, 0:C] * scale[:, 0:N, :, None]  # [B, n, p, c] heads 2p
    r1 = o[:, N:128, :, 0:C] * scale[:, N:128, :, None]  # heads 2p+1
    r = np.empty((B, N, H, C), np.float32)
    r[:, :, 0::2, :] = r0
    r[:, :, 1::2, :] = r1
    Wv_h = Wv.reshape(H, DH, C)
    vsum = np.einsum("bnhc,hdc->bnhd", r, Wv_h)  # [B, N, H, DH]
    out = vsum.reshape(B, N, C) @ Wp.T + bp
    return out.astype(np.float32)
